# revision 6
# baseline (speedup 1.0000x reference)
"""Trainium2 Bass kernel: e3nn-style GNN convolution (FC-net edge weights ->
FullyConnectedTensorProduct -> scatter-sum over edge_dst).

v2 strategy (edge-parallel, dst-sharded, DVE-optimized):
  * Host packs nodes into 160 blocks (<=128 nodes each) balancing per-block
    edge counts (greedy, degree-descending) so t_b ~= 16 tiles/block.
  * Core c owns 20 blocks. Per 128-edge tile (edges on SBUF partitions):
      - indirect-DMA gather of node_features[src] -> xg [128,64] f16
      - PE: h^T = relu(fc_w1^T @ sc^T); w = h^T.T @ fc_w2p [128,1024] (PSUM)
      - Scalar: relu + PSUM->SBUF f16 drain of w
      - DVE (batched over groups of 8 tiles, all f16 2x-mode):
        4 broadcast-mults -> prod [128,1536]; 4-level pairwise add-tree
        (instead of 1x-mode tensor_reduce); combines via batched mult/add
      - PE: scatter via host-precomputed one-hot S (DMA'd), transposed
        accumulate accT[64,128] across the block's tiles.
  * Block PSUM -> DRAM; host inverse-permutes to node order.

Weight layout: fc_w2 reordered to per-tile path order [p1|p4|p2|p3], (o,i)
with i innermost so the contraction over i is the tree's innermost dim.
All normalization constants folded into fc_w1/fc_w2 on host.
"""

import heapq
import math

import numpy as np

N_NODES = 20000
N_CORES = 8
BLK = 128
BLOCKS = 20  # per core
NBLK = N_CORES * BLOCKS
P = 128
MUL = 16
G = 8  # DVE batch group (tiles)

_CACHE: dict = {}


def _build(n_tiles: int, t_b: int, blocks: int = BLOCKS, n_nodes: int = N_NODES):
    import concourse.bass as bass
    import concourse.mybir as mybir
    import concourse.tile as tile
    from concourse import bacc

    dt = mybir.dt
    Alu = mybir.AluOpType
    Act = mybir.ActivationFunctionType

    nc = bacc.Bacc("TRN2", target_bir_lowering=False, debug=False)

    xg_tab = nc.dram_tensor("xg_tab", [n_nodes, 64], dt.float16, kind="ExternalInput")
    srcT = nc.dram_tensor("srcT", [P, n_tiles], dt.int32, kind="ExternalInput")
    attrP = nc.dram_tensor("attrP", [P, 4 * n_tiles], dt.float32, kind="ExternalInput")
    scT = nc.dram_tensor("scT", [3, n_tiles * P], dt.float16, kind="ExternalInput")
    Sg = nc.dram_tensor("Sg", [P, n_tiles * P], dt.float16, kind="ExternalInput")
    fw1 = nc.dram_tensor("fw1", [3, 256], dt.float16, kind="ExternalInput")
    fw2 = nc.dram_tensor("fw2", [256, 1024], dt.float16, kind="ExternalInput")
    outp = nc.dram_tensor("outp", [blocks * 64, P], dt.float32, kind="ExternalOutput")

    with tile.TileContext(nc) as tc:
        with (
            tc.tile_pool(name="const", bufs=1) as cp,
            tc.tile_pool(name="sb", bufs=2) as sb,
            tc.tile_pool(name="dve1", bufs=1) as dv,
            tc.tile_pool(name="scp", bufs=2) as scp,
            tc.tile_pool(name="wps", bufs=2, space="PSUM") as wps,
            tc.tile_pool(name="hps", bufs=2, space="PSUM") as hps,
            tc.tile_pool(name="aps", bufs=2, space="PSUM") as aps,
        ):
            srcT_sb = cp.tile([P, n_tiles], dt.int32)
            nc.sync.dma_start(srcT_sb[:], srcT[:])
            attr_sb = cp.tile([P, 4 * n_tiles], dt.float32)
            nc.sync.dma_start(attr_sb[:], attrP[:])
            fw1_sb = cp.tile([3, 256], dt.float16)
            nc.sync.dma_start(fw1_sb[:], fw1[:])
            fw2_sb = cp.tile([P, 2048], dt.float16)
            nc.sync.dma_start(fw2_sb[:, 0:1024], fw2[0:128, :])
            nc.sync.dma_start(fw2_sb[:, 1024:2048], fw2[128:256, :])

            for b in range(blocks):
                scc = scp.tile([3, t_b * P], dt.float16, tag="scc")
                nc.sync.dma_start(scc[:], scT[:, b * t_b * P : (b + 1) * t_b * P])
                accT = aps.tile([64, P], dt.float32, tag="accT")
                for g0 in range(0, t_b, G):
                    Gc = min(G, t_b - g0)
                    xg = sb.tile([P, G * 64], dt.float16, tag="xg")
                    Ss = sb.tile([P, G * P], dt.float16, tag="Ss")
                    nc.sync.dma_start(
                        Ss[:, 0 : Gc * P],
                        Sg[:, (b * t_b + g0) * P : (b * t_b + g0 + Gc) * P],
                    )
                    at16 = sb.tile([P, G * 4], dt.float16, tag="at16")
                    nc.scalar.activation(
                        at16[:, 0 : Gc * 4],
                        attr_sb[:, 4 * (b * t_b + g0) : 4 * (b * t_b + g0 + Gc)],
                        Act.Copy,
                    )
                    w16 = sb.tile([P, G * 1024], dt.float16, tag="w16")
                    for j in range(Gc):
                        t = b * t_b + g0 + j
                        nc.gpsimd.indirect_dma_start(
                            out=xg[:, j * 64 : (j + 1) * 64],
                            out_offset=None,
                            in_=xg_tab[:],
                            in_offset=bass.IndirectOffsetOnAxis(
                                ap=srcT_sb[:, t : t + 1], axis=0
                            ),
                        )
                        hp = hps.tile([P, 256], dt.float32, tag="hp")
                        rhs_sc = scc[:, (g0 + j) * P : (g0 + j + 1) * P]
                        nc.tensor.matmul(
                            out=hp[:, 0:128], lhsT=fw1_sb[:, 0:128], rhs=rhs_sc,
                            start=True, stop=True,
                        )
                        nc.tensor.matmul(
                            out=hp[:, 128:256], lhsT=fw1_sb[:, 128:256], rhs=rhs_sc,
                            start=True, stop=True,
                        )
                        h16 = sb.tile([P, 256], dt.float16, tag="h16")
                        nc.scalar.activation(h16[:], hp[:], Act.Relu)
                        wp = wps.tile([P, 1024], dt.float32, tag="wp")
                        for kc in range(2):
                            for nh in range(2):
                                nc.tensor.matmul(
                                    out=wp[:, nh * 512 : (nh + 1) * 512],
                                    lhsT=h16[:, kc * 128 : (kc + 1) * 128],
                                    rhs=fw2_sb[
                                        :,
                                        kc * 1024 + nh * 512 : kc * 1024 + (nh + 1) * 512,
                                    ],
                                    start=(kc == 0),
                                    stop=(kc == 1),
                                )
                        nc.scalar.activation(
                            w16[:, j * 1024 : (j + 1) * 1024], wp[:], Act.Copy
                        )

                    # ---- batched DVE tensor-product over the group ----
                    xg_r = xg[:].rearrange("p (t f) -> p t f", f=64)
                    s_v = xg_r[:, 0:Gc, 0:16]
                    v_v = xg_r[:, 0:Gc, 16:64]
                    a_r = at16[:].rearrange("p (t f) -> p t f", f=4)
                    shv = a_r[:, 0:Gc, 1:4]
                    w_r = w16[:].rearrange("p (t f) -> p t f", f=1024)

                    # dot_i = sum_c v[i,c]*shv[c] (normalized on host via w2)
                    tmp_d = dv.tile([P, G * 48], dt.float16, tag="tmp_d")
                    td = tmp_d[:].rearrange("p (t i c) -> p t i c", i=16, c=3)[:, 0:Gc]
                    nc.vector.tensor_tensor(
                        out=td,
                        in0=v_v.rearrange("p t (i c) -> p t i c", c=3),
                        in1=shv.unsqueeze(2).broadcast_to([P, Gc, 16, 3]),
                        op=Alu.mult,
                    )
                    dsc = sb.tile([P, G * 16], dt.float32, tag="dsc")
                    nc.vector.tensor_reduce(
                        out=dsc[:].rearrange("p (t i) -> p t i", i=16)[:, 0:Gc],
                        in_=td,
                        axis=mybir.AxisListType.X,
                        op=Alu.add,
                    )
                    dsc16 = sb.tile([P, G * 16], dt.float16, tag="dsc16")
                    nc.scalar.activation(
                        dsc16[:, 0 : Gc * 16], dsc[:, 0 : Gc * 16], Act.Copy
                    )
                    # v transposed (c,i) for path4 2x-mode mult
                    vT = sb.tile([P, G * 48], dt.float16, tag="vT")
                    nc.scalar.activation(
                        vT[:].rearrange("p (t c i) -> p t c i", c=3, i=16)[:, 0:Gc],
                        v_v.rearrange("p t (i c) -> p t c i", c=3),
                        Act.Copy,
                    )

                    # prod layout/tile: [p1(o,i) 256 | p4(o,c,i) 768 | p2 256 | p3 256]
                    prod = dv.tile([P, G * 1536], dt.float16, tag="prod")
                    p_r = prod[:].rearrange("p (t f) -> p t f", f=1536)
                    s_b = s_v.unsqueeze(2).broadcast_to([P, Gc, 16, 16])
                    nc.vector.tensor_tensor(
                        out=p_r[:, 0:Gc, 0:256].rearrange("p t (o i) -> p t o i", o=16),
                        in0=w_r[:, 0:Gc, 0:256].rearrange("p t (o i) -> p t o i", o=16),
                        in1=s_b,
                        op=Alu.mult,
                    )
                    # path4 mult needs 4 free dims batched (ISA max 3) -> per tile
                    for j in range(Gc):
                        nc.vector.tensor_tensor(
                            out=prod[
                                :, j * 1536 + 256 : j * 1536 + 1024
                            ].rearrange("p (o c i) -> p o c i", o=16, c=3),
                            in0=w16[:, j * 1024 + 256 : j * 1024 + 512]
                            .rearrange("p (o i) -> p o i", o=16)
                            .unsqueeze(2)
                            .broadcast_to([P, 16, 3, 16]),
                            in1=vT[:, j * 48 : (j + 1) * 48]
                            .rearrange("p (c i) -> p c i", c=3)
                            .unsqueeze(1)
                            .broadcast_to([P, 16, 3, 16]),
                            op=Alu.mult,
                        )
                    nc.vector.tensor_tensor(
                        out=p_r[:, 0:Gc, 1024:1280].rearrange(
                            "p t (o i) -> p t o i", o=16
                        ),
                        in0=w_r[:, 0:Gc, 512:768].rearrange(
                            "p t (o i) -> p t o i", o=16
                        ),
                        in1=dsc16[:]
                        .rearrange("p (t i) -> p t i", i=16)[:, 0:Gc]
                        .unsqueeze(2)
                        .broadcast_to([P, Gc, 16, 16]),
                        op=Alu.mult,
                    )
                    nc.vector.tensor_tensor(
                        out=p_r[:, 0:Gc, 1280:1536].rearrange(
                            "p t (o i) -> p t o i", o=16
                        ),
                        in0=w_r[:, 0:Gc, 768:1024].rearrange(
                            "p t (o i) -> p t o i", o=16
                        ),
                        in1=s_b,
                        op=Alu.mult,
                    )

                    # pairwise add tree over i (96 groups of 16 per tile)
                    t1 = dv.tile([P, G * 768], dt.float16, tag="t1")
                    pr16 = prod[:].rearrange("p (x i) -> p x i", i=16)
                    t1v = t1[:].rearrange("p (x i) -> p x i", i=8)
                    nc.vector.tensor_tensor(
                        out=t1v[:, 0 : 96 * Gc],
                        in0=pr16[:, 0 : 96 * Gc, 0:8],
                        in1=pr16[:, 0 : 96 * Gc, 8:16],
                        op=Alu.add,
                    )
                    t2 = dv.tile([P, G * 384], dt.float16, tag="t2")
                    t1r = t1[:].rearrange("p (x i) -> p x i", i=8)
                    t2v = t2[:].rearrange("p (x i) -> p x i", i=4)
                    nc.vector.tensor_tensor(
                        out=t2v[:, 0 : 96 * Gc],
                        in0=t1r[:, 0 : 96 * Gc, 0:4],
                        in1=t1r[:, 0 : 96 * Gc, 4:8],
                        op=Alu.add,
                    )
                    t3 = dv.tile([P, G * 192], dt.float16, tag="t3")
                    t2r = t2[:].rearrange("p (x i) -> p x i", i=4)
                    t3v = t3[:].rearrange("p (x i) -> p x i", i=2)
                    nc.vector.tensor_tensor(
                        out=t3v[:, 0 : 96 * Gc],
                        in0=t2r[:, 0 : 96 * Gc, 0:2],
                        in1=t2r[:, 0 : 96 * Gc, 2:4],
                        op=Alu.add,
                    )
                    t3g = t3[:].rearrange("p (t g i) -> p t g i", g=96, i=2)
                    M64 = dv.tile([P, G * 64], dt.float16, tag="M64")
                    nc.vector.tensor_tensor(
                        out=M64[:]
                        .rearrange("p (t g) -> p t g", g=64)[:, 0:Gc]
                        .unsqueeze(3),
                        in0=t3g[:, 0:Gc, 0:64, 0:1],
                        in1=t3g[:, 0:Gc, 0:64, 1:2],
                        op=Alu.add,
                    )
                    addB = dv.tile([P, G * 64], dt.float16, tag="addB")
                    aB_r = addB[:].rearrange("p (t f) -> p t f", f=64)
                    nc.vector.tensor_tensor(
                        out=aB_r[:, 0:Gc, 0:16].unsqueeze(3),
                        in0=t3g[:, 0:Gc, 64:80, 0:1],
                        in1=t3g[:, 0:Gc, 64:80, 1:2],
                        op=Alu.add,
                    )
                    Mr3 = dv.tile([P, G * 16], dt.float16, tag="Mr3")
                    nc.vector.tensor_tensor(
                        out=Mr3[:]
                        .rearrange("p (t g) -> p t g", g=16)[:, 0:Gc]
                        .unsqueeze(3),
                        in0=t3g[:, 0:Gc, 80:96, 0:1],
                        in1=t3g[:, 0:Gc, 80:96, 1:2],
                        op=Alu.add,
                    )
                    # tv = r3[o] * shv[c] into addB[:, 16:64]
                    nc.vector.tensor_tensor(
                        out=aB_r[:, 0:Gc, 16:64].rearrange(
                            "p t (o c) -> p t o c", c=3
                        ),
                        in0=Mr3[:]
                        .rearrange("p (t g) -> p t g", g=16)[:, 0:Gc]
                        .unsqueeze(3)
                        .broadcast_to([P, Gc, 16, 3]),
                        in1=shv.unsqueeze(2).broadcast_to([P, Gc, 16, 3]),
                        op=Alu.mult,
                    )
                    # feat = M64*sh_s + addB (shs replicated on scalar for 2x mode)
                    shs_rep = sb.tile([P, G * 64], dt.float16, tag="shs_rep")
                    nc.scalar.activation(
                        shs_rep[:].rearrange("p (t f) -> p t f", f=64)[:, 0:Gc],
                        a_r[:, 0:Gc, 0:1].broadcast_to([P, Gc, 64]),
                        Act.Copy,
                    )
                    fm = dv.tile([P, G * 64], dt.float16, tag="fm")
                    nc.vector.tensor_tensor(
                        out=fm[:].rearrange("p (t f) -> p t f", f=64)[:, 0:Gc],
                        in0=M64[:].rearrange("p (t f) -> p t f", f=64)[:, 0:Gc],
                        in1=shs_rep[:].rearrange("p (t f) -> p t f", f=64)[:, 0:Gc],
                        op=Alu.mult,
                    )
                    feat = sb.tile([P, G * 64], dt.float16, tag="feat")
                    nc.vector.tensor_tensor(
                        out=feat[:, 0 : Gc * 64],
                        in0=fm[:, 0 : Gc * 64],
                        in1=addB[:, 0 : Gc * 64],
                        op=Alu.add,
                    )

                    # scatter: accT[f, dst] += feat^T @ S
                    for j in range(Gc):
                        ti = g0 + j
                        nc.tensor.matmul(
                            out=accT[:],
                            lhsT=feat[:, j * 64 : (j + 1) * 64],
                            rhs=Ss[:, j * P : (j + 1) * P],
                            start=(ti == 0),
                            stop=(ti == t_b - 1),
                        )
                osb = sb.tile([64, P], dt.float32, tag="osb")
                nc.scalar.activation(osb[:], accT[:], Act.Copy)
                nc.sync.dma_start(outp[b * 64 : (b + 1) * 64, :], osb[:])
    nc.compile()
    return nc


def _pack_blocks(dst):
    """Greedy degree-balanced packing of nodes into NBLK blocks (<=128 nodes,
    near-equal edge counts). Returns node_block, node_slot, t_b."""
    deg = np.bincount(dst, minlength=N_NODES).astype(np.int64)
    order = np.argsort(-deg, kind="stable")
    heap = [(0, b) for b in range(NBLK)]
    heapq.heapify(heap)
    nodecnt = np.zeros(NBLK, np.int64)
    edgecnt = np.zeros(NBLK, np.int64)
    node_block = np.empty(N_NODES, np.int64)
    node_slot = np.empty(N_NODES, np.int64)
    for n in order:
        while True:
            e, b = heapq.heappop(heap)
            if nodecnt[b] < BLK:
                break
        node_block[n] = b
        node_slot[n] = nodecnt[b]
        nodecnt[b] += 1
        edgecnt[b] += deg[n]
        if nodecnt[b] < BLK:
            heapq.heappush(heap, (int(edgecnt[b]), b))
    t_b = max(1, int(math.ceil(edgecnt.max() / P)))
    return node_block, node_slot, t_b


def _prep(inputs):
    nf = np.ascontiguousarray(np.asarray(inputs["node_features"], dtype=np.float32))
    src = np.asarray(inputs["edge_src"]).astype(np.int64)
    dst = np.asarray(inputs["edge_dst"]).astype(np.int64)
    attr = np.asarray(inputs["edge_attr"], dtype=np.float32)
    sc = np.asarray(inputs["edge_scalars"], dtype=np.float32)
    w1 = np.asarray(inputs["fc_w1"], dtype=np.float32)
    w2 = np.asarray(inputs["fc_w2"], dtype=np.float32)

    fw1 = np.ascontiguousarray((w1 / np.sqrt(3.0)).astype(np.float32))
    # fc_w2 [256, (path,i,o)] -> [256, (path,o,i)], norms folded, then path
    # reorder to [p1, p4, p2, p3] (matches prod layout in the kernel)
    w2r = w2.reshape(256, 4, MUL, MUL).transpose(0, 1, 3, 2).copy()
    scale = (
        (1.0 / np.sqrt(256.0))
        * (1.0 / np.sqrt(2.0 * MUL))
        * (1.0 / np.sqrt(16.0))
    )
    w2r *= scale
    w2r[:, 1] *= 1.0 / np.sqrt(3.0)  # dot normalization (path 2 only)
    w2r = w2r[:, [0, 3, 1, 2]]
    fw2 = np.ascontiguousarray(w2r.reshape(256, 1024).astype(np.float32))

    node_block, node_slot, t_b = _pack_blocks(dst)
    n_tiles = BLOCKS * t_b
    e_pad = n_tiles * P

    eb = node_block[dst]
    order_e = np.argsort(eb, kind="stable")
    srcs = src[order_e]
    slots = node_slot[dst][order_e]
    attrs, scs = attr[order_e], sc[order_e]
    counts = np.bincount(eb, minlength=NBLK)
    seg_start = np.zeros(NBLK + 1, np.int64)
    np.cumsum(counts, out=seg_start[1:])

    nf16 = nf.astype(np.float16)
    fw1_16 = fw1.astype(np.float16)
    fw2_16 = fw2.astype(np.float16)

    in_maps = []
    for c in range(N_CORES):
        src_c = np.zeros(e_pad, np.int32)
        attr_c = np.zeros((e_pad, 4), np.float32)
        sc_c = np.zeros((e_pad, 3), np.float32)
        S_c = np.zeros((n_tiles, P, P), np.float16)
        for b in range(BLOCKS):
            g = c * BLOCKS + b
            a0, a1 = int(seg_start[g]), int(seg_start[g + 1])
            n = a1 - a0
            off = b * t_b * P
            src_c[off : off + n] = srcs[a0:a1]
            attr_c[off : off + n] = attrs[a0:a1]
            sc_c[off : off + n] = scs[a0:a1]
            kk = np.arange(n)
            S_c[b * t_b + kk // P, kk % P, slots[a0:a1]] = 1.0
        in_maps.append(
            {
                "xg_tab": nf16,
                "srcT": np.ascontiguousarray(src_c.reshape(n_tiles, P).T),
                "attrP": np.ascontiguousarray(
                    attr_c.reshape(n_tiles, P, 4)
                    .transpose(1, 0, 2)
                    .reshape(P, 4 * n_tiles)
                ),
                "scT": np.ascontiguousarray(sc_c.T.astype(np.float16)),
                "Sg": np.ascontiguousarray(
                    S_c.transpose(1, 0, 2).reshape(P, n_tiles * P)
                ),
                "fw1": fw1_16,
                "fw2": fw2_16,
            }
        )
    return in_maps, n_tiles, t_b, node_block, node_slot


def kernel(**inputs) -> np.ndarray:
    from concourse.bass_interp import get_hw_module
    from concourse.bass_utils import run_bass_kernel_spmd

    in_maps, n_tiles, t_b, node_block, node_slot = _prep(inputs)
    key = (n_tiles, t_b)
    if key not in _CACHE:
        _CACHE[key] = _build(n_tiles, t_b)
    nc = _CACHE[key]
    old = nc.m
    nc.m = get_hw_module(nc.m)
    try:
        res = run_bass_kernel_spmd(nc, in_maps, core_ids=list(range(N_CORES)))
    finally:
        nc.m = old
    full = np.empty((N_NODES, 64), np.float32)
    core_of = node_block // BLOCKS
    rowidx = (node_block % BLOCKS) * P + node_slot
    for c in range(N_CORES):
        rows = (
            res.results[c]["outp"]
            .reshape(BLOCKS, 64, P)
            .transpose(0, 2, 1)
            .reshape(BLOCKS * P, 64)
        )
        m = core_of == c
        full[m] = rows[rowidx[m]]
    return np.ascontiguousarray(full)
